# revision 9
# baseline (speedup 1.0000x reference)
"""Trainium2 Bass kernel for nn_CausalSTDiT2Block (spatial-temporal DiT block).

8 cores = 4 batches x 2 shards. Phase A (spatial attention) shards each batch
by t-half; a pairwise AllGather exchanges the residual stream; Phase B
(temporal+cross attention, MLP) shards by s-half, selected from the gathered
buffer with a per-core 0/1 mask so the SPMD program stays uniform.

Host folds AdaLN modulate/gate into per-batch weights, precomputes cross-attn
K/V from y, de-interleaves + pads temporal q/k head dims (36->64) and spatial
heads (72->96 / cross 72->128) so every attention matmul sits at a 32-aligned
partition base. Softmax skips max subtraction (scores are O(1), f32-safe).
Tokens are processed in 512-token quarters to bound SBUF usage.
"""
import numpy as np
import ml_dtypes

import concourse.bass as bass
import concourse.mybir as mybir
import concourse.tile as tile
from concourse import bacc
from concourse.bass_utils import run_bass_kernel_spmd
from concourse.masks import make_identity

P = 128
C = 1152
NCT = C // P            # 9
NH = 16
HD = 72
HHD = 36
B = 4
T = 16
S = 256
N = T * S
LY = 120
TOK = 2048
Q = 512                 # tokens per quarter
NQT = Q // P            # 4 tok tiles per quarter
SCALE = HD ** -0.5
AF = NH * (HD + 1)      # 1168 (v-aug with ones col per head)
QKS_F = 2 * NH * 96     # 3072 spatial qk padded features (96/head)
QKT_F = 4 * NH * 64     # 4096 temporal de-interleaved padded (64/half-head)
QC_F = NH * P           # 2048 cross q padded (128/head)
FT3 = ((0, 384), (384, 384), (768, 384))

bf = mybir.dt.bfloat16
f32 = mybir.dt.float32
AF_T = mybir.ActivationFunctionType
ALU = mybir.AluOpType
bf16np = ml_dtypes.bfloat16

_CACHED_NC = None


def _mm_segs(row0, nrows):
    """Split rows [row0, row0+nrows) into matmul-legal (tile, p0, n) pieces:
    base 0 -> up to 128, base 64 -> up to 64, base 32/96 -> up to 32."""
    out = []
    r = row0
    end = row0 + nrows
    while r < end:
        t, b = divmod(r, P)
        if b == 0:
            take = min(end - r, P)
        elif b % 64 == 0:
            take = min(end - r, 64)
        elif b % 32 == 0:
            take = min(end - r, 32)
        else:
            raise AssertionError(f"unaligned base {b}")
        out.append((t, b, take))
        r += take
    return out


def build_nc(debug=False, nocc=False):
    nc = bacc.Bacc(None, target_bir_lowering=False)
    dbg = {}

    def dbg_out(name, shape, dtype=bf):
        if name not in dbg:
            dbg[name] = nc.dram_tensor(f"dbg_{name}", list(shape), dtype,
                                       kind="ExternalOutput")
        return dbg[name]

    def di(name, shape, dtype):
        return nc.dram_tensor(name, list(shape), dtype, kind="ExternalInput")

    xa_d = di("xa", [TOK, C], f32)
    wqk_s_d = di("wqk_s", [C, QKS_F], bf)
    bqk_s_d = di("bqk_s_col", [P, QKS_F // P], f32)
    wv_s_d = di("wv_s", [C, C], bf)
    bv_s_d = di("bv_s_row", [1, C], bf)
    wproj_s_d = di("wproj_s", [C, C], bf)
    bproj_s_d = di("bproj_s_row", [1, C], bf)
    wqk_t_d = di("wqk_t_de", [C, QKT_F], bf)
    bqk_t_d = di("bqk_t_col", [P, QKT_F // P], f32)
    wv_t_d = di("wv_t", [C, C], bf)
    bv_t_d = di("bv_t_row", [1, C], bf)
    wproj_t_d = di("wproj_t", [C, C], bf)
    bproj_t_d = di("bproj_t_row", [1, C], bf)
    wq_c_d = di("wq_c", [C, QC_F], bf)
    bq_c_d = di("bq_c_col", [P, QC_F // P], f32)
    kct_d = di("k_ct_pad", [P, NH, LY], bf)
    vca_d = di("v_c_aug", [LY, AF], bf)
    wo_c_d = di("wo_c", [C, C], bf)
    bo_c_d = di("bo_c_row", [1, C], bf)
    w1_d = di("w1", [C, 4 * C], bf)
    b1_d = di("b1_col", [P, 36], f32)
    w2_d = di("w2", [4 * C, C], bf)
    b2_d = di("b2_row", [1, C], bf)
    cosT_d = di("cosT", [P, 8, Q], bf)
    sinT_d = di("sinT", [P, 8, Q], bf)
    mask_d = di("mask", [P, P], bf)
    msel_d = di("msel", [P, 2], f32)
    out_d = nc.dram_tensor("out", [TOK, C], f32, kind="ExternalOutput")

    with tile.TileContext(nc) as tc:
        with (
            tc.tile_pool(name="const", bufs=1) as cpool,
            tc.tile_pool(name="dram", bufs=1, space="DRAM") as dram,
            tc.tile_pool(name="lnp", bufs=1) as lnp,
            tc.tile_pool(name="spool", bufs=1) as spool,
            tc.tile_pool(name="rpool", bufs=1) as rpool,
            tc.tile_pool(name="wpool", bufs=1) as wpool,
            tc.tile_pool(name="big", bufs=1) as big,
            tc.tile_pool(name="tp_ps", bufs=1, space="PSUM") as tp_ps,
            tc.tile_pool(name="mm_ps", bufs=1, space="PSUM") as mm_ps,
            tc.tile_pool(name="at_ps", bufs=1, space="PSUM") as at_ps,
        ):
            ident = cpool.tile([P, P], bf, tag="ident")
            make_identity(nc, ident)
            ones1 = cpool.tile([1, P], bf, tag="ones1")
            nc.gpsimd.memset(ones1[:], 1.0)
            mask_sb = cpool.tile([P, P], bf, tag="mask")
            nc.sync.dma_start(mask_sb[:], mask_d[:])
            msel_sb = cpool.tile([P, 2], f32, tag="msel")
            nc.sync.dma_start(msel_sb[:], msel_d[:])
            kct_sb = cpool.tile([P, NH, LY], bf, tag="kct")
            nc.sync.dma_start(kct_sb[:], kct_d[:])
            vca_sb = cpool.tile([LY, AF], bf, tag="vca")
            nc.sync.dma_start(vca_sb[:], vca_d[:])
            bqk_s_sb = cpool.tile([P, QKS_F // P], f32, tag="bqks")
            nc.sync.dma_start(bqk_s_sb[:], bqk_s_d[:])
            bqk_t_sb = cpool.tile([P, QKT_F // P], f32, tag="bqkt")
            nc.sync.dma_start(bqk_t_sb[:], bqk_t_d[:])
            bq_c_sb = cpool.tile([P, QC_F // P], f32, tag="bqc")
            nc.sync.dma_start(bq_c_sb[:], bq_c_d[:])
            b1_sb = cpool.tile([P, 36], f32, tag="b1")
            nc.sync.dma_start(b1_sb[:], b1_d[:])
            brows = {}
            for nm, d in [("bv_s", bv_s_d), ("bproj_s", bproj_s_d),
                          ("bv_t", bv_t_d), ("bproj_t", bproj_t_d),
                          ("bo_c", bo_c_d), ("b2", b2_d)]:
                br = cpool.tile([1, C], bf, name=f"brow_{nm}", tag=f"brow_{nm}")
                nc.sync.dma_start(br[:], d[:])
                brows[nm] = br

            eps_sb = cpool.tile([P, 1], f32, tag="eps")
            nc.gpsimd.memset(eps_sb[:], 1e-6)

            ag_src = dram.tile([TOK, C], f32, tag="ag_src")
            ag_dst = dram.tile([2 * TOK, C], f32, tag="ag_dst")
            xb_dram = dram.tile([TOK, C], f32, tag="xb")
            x3_dram = dram.tile([TOK, C], f32, tag="x3")
            x4_dram = dram.tile([TOK, C], f32, tag="x4")

            # ---------------- helpers ----------------
            def transpose_block(src_fn, dst_T, lt, njt, f_base=0):
                for j in range(njt):
                    ps = tp_ps.tile([P, P], bf, name="tps", tag="tps", bufs=2)
                    nc.tensor.transpose(ps[:], src_fn(j), ident[:])
                    nc.vector.tensor_copy(
                        dst_T[:, f_base + j, lt * P:(lt + 1) * P], ps[:])

            def ln_apply(xt_ap, dst_ap):
                bn6 = lnp.tile([P, 3, 6], f32, name="bn6", tag="bn6", bufs=2)
                for a in range(3):
                    nc.vector.bn_stats(bn6[:, a, :], xt_ap[:, a * 384:(a + 1) * 384])
                mv = lnp.tile([P, 2], f32, name="mv", tag="mv", bufs=2)
                nc.vector.bn_aggr(mv[:], bn6[:])
                std = lnp.tile([P, 1], f32, name="std", tag="std", bufs=2)
                nc.scalar.activation(std[:], mv[:, 1:2], AF_T.Sqrt, bias=eps_sb[:])
                rstd = lnp.tile([P, 1], f32, name="rstd", tag="rstd", bufs=2)
                nc.vector.reciprocal(rstd[:], std[:])
                nc.vector.tensor_scalar(dst_ap, xt_ap, mv[:, 0:1], rstd[:],
                                        op0=ALU.subtract, op1=ALU.mult)

            def gemm_wst(w_dram, nft, bcol, rhs_T, dst_T):
                for ft in range(nft):
                    wc = wpool.tile([P, NCT, P], bf, name="wcol", tag="wcol",
                                    bufs=3)
                    nc.sync.dma_start(
                        wc[:], w_dram[:, ft * P:(ft + 1) * P]
                        .rearrange("(a p) f -> p a f", p=P))
                    psum = mm_ps.tile([P, Q], f32, name="gpsum", tag="gpsum",
                                      bufs=2)
                    for c in range(NCT):
                        nc.tensor.matmul(psum[:], wc[:, c, :], rhs_T[:, c, :],
                                         start=(c == 0), stop=(c == NCT - 1))
                    nc.scalar.activation(dst_T[:, ft, :], psum[:], AF_T.Identity,
                                         bias=bcol[:, ft:ft + 1])

            def gemm_ast(w_dram, brow, lhsT_T, epi, ftiles=FT3):
                for f0, fn in ftiles:
                    wa = wpool.tile([P, NCT, 432], bf, name="wast", tag="wast",
                                    bufs=2)
                    nc.sync.dma_start(
                        wa[:, :, :fn], w_dram[:, f0:f0 + fn]
                        .rearrange("(a p) f -> p a f", p=P))
                    for lt in range(NQT):
                        psum = mm_ps.tile([P, Q], f32, name="gpsum", tag="gpsum",
                                          bufs=2)
                        for c in range(NCT):
                            nc.tensor.matmul(psum[:, :fn],
                                             lhsT_T[:, c, lt * P:(lt + 1) * P],
                                             wa[:, c, :fn],
                                             start=(c == 0), stop=False)
                        nc.tensor.matmul(psum[:, :fn], ones1[:1, :],
                                         brow[:1, f0:f0 + fn],
                                         start=False, stop=True)
                        epi(psum, lt, f0, fn)

            def gemm_vaug(w_dram, brow, lhsT_T, vaug):
                for lt in range(NQT):
                    nc.gpsimd.memset(
                        vaug[:, lt, :].rearrange("p (h x) -> p h x", x=HD + 1)
                        [:, :, HD:], 1.0)

                def epi(psum, lt, f0, fn):
                    h0 = f0 // HD
                    nh = fn // HD
                    dst = vaug[:, lt, :].rearrange("p (h x) -> p h x", x=HD + 1)
                    nc.scalar.activation(
                        dst[:, h0:h0 + nh, :HD],
                        psum[:, :fn].rearrange("p (h x) -> p h x", x=HD),
                        AF_T.Copy)

                gemm_ast(w_dram, brow, lhsT_T, epi,
                         ftiles=((0, 432), (432, 432), (864, 288)))

            def normalize(attn, sums):
                for lt in range(NQT):
                    rs = spool.tile([P, NH], f32, name="rs", tag="rs", bufs=2)
                    nc.vector.reciprocal(rs[:], sums[:, lt, :])
                    for h in range(NH):
                        sl = slice(h * HD, (h + 1) * HD)
                        nc.vector.tensor_scalar_mul(attn[:, lt, sl],
                                                    attn[:, lt, sl],
                                                    rs[:, h:h + 1])

            def evict_av(ps_av, attn, sums, lt, h):
                nc.scalar.activation(attn[:, lt, h * HD:(h + 1) * HD],
                                     ps_av[:, :HD], AF_T.Copy)
                nc.scalar.activation(sums[:, lt, h:h + 1],
                                     ps_av[:, HD:HD + 1], AF_T.Copy)

            def seg_matmul(psum_ap, segs_mm, ltile_fn, rtile_fn):
                for i, (jj, p0, n) in enumerate(segs_mm):
                    tp = (p0, 0) if p0 == 96 else None
                    nc.tensor.matmul(psum_ap,
                                     ltile_fn(jj)[p0:p0 + n],
                                     rtile_fn(jj)[p0:p0 + n],
                                     tile_position=tp,
                                     start=(i == 0), stop=(i == len(segs_mm) - 1))

            # ==================== PHASE A ====================
            for qq in range(4):
                featT = big.tile([P, NCT, Q], bf, name="xhat1T", tag="featT")
                for lt in range(NQT):
                    ti = qq * NQT + lt
                    xt = lnp.tile([P, C], f32, name="xt", tag="xt", bufs=2)
                    nc.sync.dma_start(xt[:], xa_d[ti * P:(ti + 1) * P, :])
                    xh = lnp.tile([P, C], bf, name="xh", tag="xh", bufs=2)
                    ln_apply(xt[:], xh[:])
                    transpose_block(lambda j, _x=xh: _x[:, j * P:(j + 1) * P],
                                    featT, lt, NCT)
                if debug and qq == 0:
                    nc.sync.dma_start(dbg_out("xhat1T", [P, NCT, Q])[:], featT[:])
                vaug = big.tile([P, NQT, AF], bf, name="vaug_s", tag="vaugT")
                gemm_vaug(wv_s_d, brows["bv_s"], featT, vaug)
                qkT = big.tile([P, QKS_F // P, Q], bf, name="qkT_s", tag="qkT")
                gemm_wst(wqk_s_d, QKS_F // P, bqk_s_sb, featT, qkT)
                if debug and qq == 0:
                    nc.sync.dma_start(dbg_out("vaug_s", [P, NQT, AF])[:], vaug[:])
                    nc.sync.dma_start(dbg_out("qkT_s", [P, QKS_F // P, Q])[:], qkT[:])
                attn = big.tile([P, NQT, C], bf, name="attn_s", tag="attn")
                sums = big.tile([P, NQT, NH], f32, name="sums_s", tag="sums")
                for w in range(2):
                    qsl = slice(w * 256, (w + 1) * 256)
                    for h in range(NH):
                        segs_mm = _mm_segs(h * 96, HD)
                        ps_sc = at_ps.tile([P, 2, 256], f32, name="ps_sc",
                                           tag="aps", bufs=2)
                        for kh in range(2):
                            ksl = slice(w * 256 + kh * P, w * 256 + (kh + 1) * P)
                            seg_matmul(
                                ps_sc[:, kh, :], segs_mm,
                                lambda jj: qkT[:, 12 + jj, ksl],
                                lambda jj: qkT[:, jj, qsl])
                        expS = spool.tile([P, 2, 256], bf, name="expS",
                                          tag="expS", bufs=3)
                        for kh in range(2):
                            nc.scalar.activation(expS[:, kh, :], ps_sc[:, kh, :],
                                                 AF_T.Exp, scale=SCALE)
                        for qh in range(2):
                            lt = w * 2 + qh
                            ps_av = at_ps.tile([P, HD + 1], f32, name="ps_av",
                                               tag="ps_av", bufs=2)
                            for kh in range(2):
                                nc.tensor.matmul(
                                    ps_av[:],
                                    expS[:, kh, qh * P:(qh + 1) * P],
                                    vaug[:, w * 2 + kh,
                                         h * (HD + 1):(h + 1) * (HD + 1)],
                                    start=(kh == 0), stop=(kh == 1))
                            evict_av(ps_av, attn, sums, lt, h)
                normalize(attn, sums)
                if debug and qq == 0:
                    nc.sync.dma_start(dbg_out("attn_s", [P, NQT, C])[:], attn[:])
                    nc.sync.dma_start(dbg_out("sums_s", [P, NQT, NH], f32)[:], sums[:])
                attnT = big.tile([P, NCT, Q], bf, name="attnT_s", tag="vaugT")
                for lt in range(NQT):
                    transpose_block(
                        lambda j, _lt=lt: attn[:, _lt, j * P:(j + 1) * P],
                        attnT, lt, NCT)

                def ep_proj_s(psum, lt, f0, fn, _qq=qq):
                    ti = _qq * NQT + lt
                    res = rpool.tile([P, 384], f32, name="res", tag="res", bufs=3)
                    nc.sync.dma_start(res[:, :fn],
                                      xa_d[ti * P:(ti + 1) * P, f0:f0 + fn])
                    x2t = rpool.tile([P, 384], f32, name="x2t", tag="x2t", bufs=3)
                    nc.vector.tensor_add(x2t[:, :fn], psum[:, :fn], res[:, :fn])
                    nc.sync.dma_start(ag_src[ti * P:(ti + 1) * P, f0:f0 + fn],
                                      x2t[:, :fn])

                gemm_ast(wproj_s_d, brows["bproj_s"], attnT, ep_proj_s)

            # ==================== EXCHANGE ====================
            if nocc:
                nc.sync.dma_start(ag_dst[:TOK], ag_src[:])
                nc.sync.dma_start(ag_dst[TOK:], ag_src[:])
            else:
                nc.gpsimd.collective_compute(
                    "AllGather", ALU.bypass,
                    replica_groups=[[0, 1], [2, 3], [4, 5], [6, 7]],
                    ins=[ag_src.opt()], outs=[ag_dst.opt()])
            ag_v = ag_dst[:].rearrange("(t s) c -> s t c", t=T)
            if debug:
                nc.sync.dma_start(dbg_out("ag_src", [TOK, C], f32)[:], ag_src[:])
                nc.sync.dma_start(dbg_out("ag_dst", [2 * TOK, C], f32)[:], ag_dst[:])

            # ==================== PHASE B ====================
            for qq in range(4):
                featT = big.tile([P, NCT, Q], bf, name="xBT", tag="featT")
                for lt in range(NQT):
                    s0 = qq * 32 + lt * 8
                    xt = lnp.tile([P, C], f32, name="xt", tag="xt", bufs=2)
                    nc.sync.dma_start(xt[:], ag_v[s0:s0 + 8])
                    xtb = lnp.tile([P, C], f32, name="xtb", tag="xtb", bufs=2)
                    nc.sync.dma_start(xtb[:], ag_v[P + s0:P + s0 + 8])
                    nc.vector.tensor_scalar_mul(xt[:], xt[:], msel_sb[:, 0:1])
                    nc.vector.tensor_scalar_mul(xtb[:], xtb[:], msel_sb[:, 1:2])
                    nc.vector.tensor_add(xt[:], xt[:], xtb[:])
                    ti = qq * NQT + lt
                    nc.sync.dma_start(xb_dram[ti * P:(ti + 1) * P, :], xt[:])
                    xh = lnp.tile([P, C], bf, name="xh", tag="xh", bufs=2)
                    nc.vector.tensor_copy(xh[:], xt[:])
                    transpose_block(lambda j, _x=xh: _x[:, j * P:(j + 1) * P],
                                    featT, lt, NCT)
                if debug and qq == 0:
                    nc.sync.dma_start(dbg_out("xBT", [P, NCT, Q])[:], featT[:])
                vaug = big.tile([P, NQT, AF], bf, name="vaug_t", tag="vaugT")
                gemm_vaug(wv_t_d, brows["bv_t"], featT, vaug)
                qkT = big.tile([P, QKT_F // P, Q], bf, name="qkT_t", tag="qkT")
                gemm_wst(wqk_t_d, QKT_F // P, bqk_t_sb, featT, qkT)
                # RoPE: blocks q_e 0..7, q_o 8..15, k_e 16..23, k_o 24..31
                for j in range(8):
                    csj = spool.tile([P, Q], bf, name="csj", tag="csj", bufs=2)
                    nc.sync.dma_start(csj[:], cosT_d[:, j, :])
                    snj = spool.tile([P, Q], bf, name="snj", tag="snj", bufs=2)
                    nc.sync.dma_start(snj[:], sinT_d[:, j, :])
                    for base in (0, 16):
                        e = qkT[:, base + j, :]
                        o = qkT[:, base + 8 + j, :]
                        t1 = spool.tile([P, Q], bf, name="rt1", tag="rt1", bufs=2)
                        t2 = spool.tile([P, Q], bf, name="rt2", tag="rt2", bufs=2)
                        t3 = spool.tile([P, Q], bf, name="rt3", tag="rt3", bufs=2)
                        t4 = spool.tile([P, Q], bf, name="rt4", tag="rt4", bufs=2)
                        nc.vector.tensor_mul(t1[:], e, csj[:])
                        nc.vector.tensor_mul(t2[:], o, snj[:])
                        nc.vector.tensor_mul(t3[:], e, snj[:])
                        nc.vector.tensor_mul(t4[:], o, csj[:])
                        nc.vector.tensor_sub(e, t1[:], t2[:])
                        nc.vector.tensor_add(o, t3[:], t4[:])
                if debug and qq == 0:
                    nc.sync.dma_start(dbg_out("qkT_t", [P, QKT_F // P, Q])[:], qkT[:])
                    nc.sync.dma_start(dbg_out("vaug_t", [P, NQT, AF])[:], vaug[:])
                attn = big.tile([P, NQT, C], bf, name="attn_t", tag="attn")
                sums = big.tile([P, NQT, NH], f32, name="sums_t", tag="sums")
                for g in range(NQT):
                    gsl = slice(g * P, (g + 1) * P)
                    for h in range(NH):
                        jj = h // 2
                        p0 = (h % 2) * 64
                        ps_sc = at_ps.tile([P, P], f32, name="ps_sct", tag="aps",
                                           bufs=2)
                        nc.tensor.matmul(ps_sc[:],
                                         qkT[p0:p0 + HHD, 16 + jj, gsl],
                                         qkT[p0:p0 + HHD, jj, gsl],
                                         start=True, stop=False)
                        nc.tensor.matmul(ps_sc[:],
                                         qkT[p0:p0 + HHD, 24 + jj, gsl],
                                         qkT[p0:p0 + HHD, 8 + jj, gsl],
                                         start=False, stop=True)
                        et = spool.tile([P, P], bf, name="et", tag="et", bufs=3)
                        nc.scalar.activation(et[:], ps_sc[:], AF_T.Exp,
                                             scale=SCALE)
                        expS = spool.tile([P, P], bf, name="expSt", tag="expSt",
                                          bufs=3)
                        nc.vector.tensor_mul(expS[:], et[:], mask_sb[:])
                        ps_av = at_ps.tile([P, HD + 1], f32, name="ps_av",
                                           tag="ps_av", bufs=2)
                        nc.tensor.matmul(
                            ps_av[:], expS[:],
                            vaug[:, g, h * (HD + 1):(h + 1) * (HD + 1)],
                            start=True, stop=True)
                        evict_av(ps_av, attn, sums, g, h)
                normalize(attn, sums)
                if debug and qq == 0:
                    nc.sync.dma_start(dbg_out("attn_t", [P, NQT, C])[:], attn[:])
                attnT = big.tile([P, NCT, Q], bf, name="attnT_t", tag="vaugT")
                for lt in range(NQT):
                    transpose_block(
                        lambda j, _lt=lt: attn[:, _lt, j * P:(j + 1) * P],
                        attnT, lt, NCT)
                x3T = big.tile([P, NCT, Q], bf, name="x3T", tag="featT")

                def ep_proj_t(psum, lt, f0, fn, _qq=qq, _x3T=x3T):
                    ti = _qq * NQT + lt
                    res = rpool.tile([P, 384], f32, name="res", tag="res", bufs=3)
                    nc.sync.dma_start(res[:, :fn],
                                      xb_dram[ti * P:(ti + 1) * P, f0:f0 + fn])
                    x3t = rpool.tile([P, 384], f32, name="x2t", tag="x2t", bufs=3)
                    nc.vector.tensor_add(x3t[:, :fn], psum[:, :fn], res[:, :fn])
                    nc.sync.dma_start(x3_dram[ti * P:(ti + 1) * P, f0:f0 + fn],
                                      x3t[:, :fn])
                    x3b = rpool.tile([P, 384], bf, name="x3b", tag="x3b", bufs=2)
                    nc.vector.tensor_copy(x3b[:, :fn], x3t[:, :fn])
                    transpose_block(lambda j, _x=x3b: _x[:, j * P:(j + 1) * P],
                                    _x3T, lt, fn // P, f_base=f0 // P)

                gemm_ast(wproj_t_d, brows["bproj_t"], attnT, ep_proj_t)
                qcT = big.tile([P, NH, Q], bf, name="qcT", tag="qkT")
                gemm_wst(wq_c_d, NH, bq_c_sb, x3T, qcT)
                if debug and qq == 0:
                    nc.sync.dma_start(dbg_out("qcT", [P, NH, Q])[:], qcT[:])
                attn = big.tile([P, NQT, C], bf, name="attn_c", tag="attn")
                sums = big.tile([P, NQT, NH], f32, name="sums_c", tag="sums")
                for h in range(NH):
                    ps_sc = at_ps.tile([LY, Q], f32, name="ps_scc", tag="aps",
                                       bufs=2)
                    nc.tensor.matmul(ps_sc[:], kct_sb[:, h, :], qcT[:, h, :],
                                     start=True, stop=True)
                    expS = spool.tile([LY, Q], bf, name="expSc", tag="expSc",
                                      bufs=3)
                    nc.scalar.activation(expS[:], ps_sc[:], AF_T.Exp, scale=SCALE)
                    for lt in range(NQT):
                        ps_av = at_ps.tile([P, HD + 1], f32, name="ps_av",
                                           tag="ps_av", bufs=2)
                        nc.tensor.matmul(
                            ps_av[:], expS[:, lt * P:(lt + 1) * P],
                            vca_sb[:LY, h * (HD + 1):(h + 1) * (HD + 1)],
                            start=True, stop=True)
                        evict_av(ps_av, attn, sums, lt, h)
                normalize(attn, sums)
                if debug and qq == 0:
                    nc.sync.dma_start(dbg_out("attn_c", [P, NQT, C])[:], attn[:])
                attnT = big.tile([P, NCT, Q], bf, name="attnT_c", tag="vaugT")
                for lt in range(NQT):
                    transpose_block(
                        lambda j, _lt=lt: attn[:, _lt, j * P:(j + 1) * P],
                        attnT, lt, NCT)

                def ep_wo_c(psum, lt, f0, fn, _qq=qq):
                    ti = _qq * NQT + lt
                    res = rpool.tile([P, 384], f32, name="res", tag="res", bufs=3)
                    nc.sync.dma_start(res[:, :fn],
                                      x3_dram[ti * P:(ti + 1) * P, f0:f0 + fn])
                    x4t = rpool.tile([P, 384], f32, name="x2t", tag="x2t", bufs=3)
                    nc.vector.tensor_add(x4t[:, :fn], psum[:, :fn], res[:, :fn])
                    nc.sync.dma_start(x4_dram[ti * P:(ti + 1) * P, f0:f0 + fn],
                                      x4t[:, :fn])

                gemm_ast(wo_c_d, brows["bo_c"], attnT, ep_wo_c)
                # LN2 -> xhat2_T
                xh2T = big.tile([P, NCT, Q], bf, name="xh2T", tag="featT")
                for lt in range(NQT):
                    ti = qq * NQT + lt
                    xt = lnp.tile([P, C], f32, name="xt", tag="xt", bufs=2)
                    nc.sync.dma_start(xt[:], x4_dram[ti * P:(ti + 1) * P, :])
                    xh = lnp.tile([P, C], bf, name="xh", tag="xh", bufs=2)
                    ln_apply(xt[:], xh[:])
                    transpose_block(lambda j, _x=xh: _x[:, j * P:(j + 1) * P],
                                    xh2T, lt, NCT)
                if debug and qq == 0:
                    nc.sync.dma_start(dbg_out("xh2T", [P, NCT, Q])[:], xh2T[:])
                # MLP
                hT = big.tile([P, 36, Q], bf, name="hT", tag="qkT")
                for ft in range(36):
                    wc = wpool.tile([P, NCT, P], bf, name="wcol", tag="wcol",
                                    bufs=3)
                    nc.sync.dma_start(
                        wc[:], w1_d[:, ft * P:(ft + 1) * P]
                        .rearrange("(a p) f -> p a f", p=P))
                    psum = mm_ps.tile([P, Q], f32, name="gpsum", tag="gpsum",
                                      bufs=2)
                    for c in range(NCT):
                        nc.tensor.matmul(psum[:], wc[:, c, :], xh2T[:, c, :],
                                         start=(c == 0), stop=(c == NCT - 1))
                    nc.scalar.activation(hT[:, ft, :], psum[:],
                                         AF_T.Gelu_apprx_tanh,
                                         bias=b1_sb[:, ft:ft + 1])
                for ct in range(3):
                    psums = [
                        at_ps.tile([P, 384], f32, name=f"fps{i}", tag="aps",
                                   bufs=2) for i in range(2)
                    ] + [
                        at_ps.tile([P, 384], f32, name=f"fps{i+2}", tag="ps_av",
                                   bufs=2) for i in range(2)
                    ]
                    for fg in range(4):
                        wa = wpool.tile([P, NCT, 432], bf, name="wast",
                                        tag="wast", bufs=2)
                        nc.sync.dma_start(
                            wa[:, :, :384],
                            w2_d[fg * C:(fg + 1) * C, ct * 384:(ct + 1) * 384]
                            .rearrange("(a p) f -> p a f", p=P))
                        for lt in range(NQT):
                            psum = psums[lt]
                            for c in range(NCT):
                                nc.tensor.matmul(
                                    psum[:],
                                    hT[:, fg * NCT + c, lt * P:(lt + 1) * P],
                                    wa[:, c, :384],
                                    start=(fg == 0 and c == 0), stop=False)
                            if fg == 3:
                                nc.tensor.matmul(
                                    psum[:], ones1[:1, :],
                                    brows["b2"][:1, ct * 384:(ct + 1) * 384],
                                    start=False, stop=True)
                    for lt in range(NQT):
                        ti = qq * NQT + lt
                        res = rpool.tile([P, 384], f32, name="res", tag="res",
                                         bufs=3)
                        nc.sync.dma_start(
                            res[:], x4_dram[ti * P:(ti + 1) * P,
                                            ct * 384:(ct + 1) * 384])
                        ot = rpool.tile([P, 384], f32, name="x2t", tag="x2t",
                                        bufs=3)
                        nc.vector.tensor_add(ot[:], psums[lt][:], res[:])
                        nc.sync.dma_start(
                            out_d[ti * P:(ti + 1) * P, ct * 384:(ct + 1) * 384],
                            ot[:])
            if debug:
                nc.sync.dma_start(dbg_out("xb", [TOK, C], f32)[:], xb_dram[:])
                nc.sync.dma_start(dbg_out("x3", [TOK, C], f32)[:], x3_dram[:])
                nc.sync.dma_start(dbg_out("x4", [TOK, C], f32)[:], x4_dram[:])
    nc.compile()
    return nc


# ==================== HOST SIDE ====================

def _host_precompute(inputs):
    fnp = np.float32
    x = np.ascontiguousarray(np.asarray(inputs['x'], fnp))
    y = np.asarray(inputs['y'], fnp)
    tt = np.asarray(inputs['t'], fnp)
    sst = np.asarray(inputs['scale_shift_table'], fnp)
    ss = sst[None] + tt.reshape(B, 6, C)
    (shift_msa, scale_msa, gate_msa,
     shift_mlp, scale_mlp, gate_mlp) = [ss[:, i] for i in range(6)]

    g = lambda k: np.asarray(inputs[k], fnp)
    w_qkv_s, b_qkv_s = g('w_qkv_s'), g('b_qkv_s')
    w_proj_s, b_proj_s = g('w_proj_s'), g('b_proj_s')
    w_qkv_t, b_qkv_t = g('w_qkv_t'), g('b_qkv_t')
    w_proj_t, b_proj_t = g('w_proj_t'), g('b_proj_t')
    wq_c, bq_c = g('wq_c'), g('bq_c')
    wkv_c, bkv_c = g('wkv_c'), g('bkv_c')
    wo_c, bo_c = g('wo_c'), g('bo_c')
    w_fc1, b_fc1 = g('w_fc1'), g('b_fc1')
    w_fc2, b_fc2 = g('w_fc2'), g('b_fc2')
    cos, sin = g('freqs_cos'), g('freqs_sin')

    def col_layout(b_vec):
        F = b_vec.shape[0]
        return np.ascontiguousarray(b_vec.reshape(F // P, P).T).astype(fnp)

    def pad_heads(W, b_vec, width, hd=HD):
        """Pad per-head blocks of (C, NH*hd) cols to `width` cols per head."""
        Wh = W.reshape(W.shape[0], NH, hd)
        Wp = np.zeros((W.shape[0], NH, width), fnp)
        Wp[:, :, :hd] = Wh
        bh = b_vec.reshape(NH, hd)
        bp = np.zeros((NH, width), fnp)
        bp[:, :hd] = bh
        return Wp.reshape(W.shape[0], NH * width), bp.reshape(NH * width)

    def de(Wb):
        """(C, NH*HD) -> even/odd halves (C, NH, 36) each."""
        Wh = Wb.reshape(-1, NH, HD)
        return Wh[:, :, 0::2], Wh[:, :, 1::2]

    # temporal: de-interleave then pad 36->64 per half-head; block order
    # [q_e | q_o | k_e | k_o], each NH*64 = 1024 cols
    def tpad(Whalf):  # (C, NH, 36) -> (C, NH*64)
        Wp = np.zeros((Whalf.shape[0], NH, 64), fnp)
        Wp[:, :, :HHD] = Whalf
        return Wp.reshape(-1, NH * 64)

    def tpad_vec(vhalf):  # (NH, 36) -> (NH*64,)
        vp = np.zeros((NH, 64), fnp)
        vp[:, :HHD] = vhalf
        return vp.reshape(-1)

    # RoPE tables in padded-64 row space: row h*64+ii -> cos[t(r), ii]
    tid = np.arange(Q) % T
    ii = np.arange(64) % HHD      # pad rows get garbage cols; zeroed below
    cosT = np.zeros((NH * 64, Q), fnp)
    sinT = np.zeros((NH * 64, Q), fnp)
    base_tab_c = cos[:T][tid][:, ii].T    # (64, Q)
    base_tab_s = sin[:T][tid][:, ii].T
    base_tab_c[HHD:] = 0
    base_tab_s[HHD:] = 0
    for h in range(NH):
        cosT[h * 64:(h + 1) * 64] = base_tab_c
        sinT[h * 64:(h + 1) * 64] = base_tab_s

    Mwin = np.zeros((P, P), fnp)
    for w in range(8):
        for kt in range(T):
            Mwin[w * T + kt, w * T + kt:(w + 1) * T] = 1.0

    per_batch = []
    for b in range(B):
        d = {}
        Wq_s = (1.0 + scale_msa[b])[:, None] * w_qkv_s[:, 0:C]
        Wk_s = (1.0 + scale_msa[b])[:, None] * w_qkv_s[:, C:2 * C]
        bq_s = shift_msa[b] @ w_qkv_s[:, 0:C] + b_qkv_s[0:C]
        bk_s = shift_msa[b] @ w_qkv_s[:, C:2 * C] + b_qkv_s[C:2 * C]
        Wqp, bqp = pad_heads(Wq_s, bq_s, 96)
        Wkp, bkp = pad_heads(Wk_s, bk_s, 96)
        d['wqk_s'] = np.concatenate([Wqp, Wkp], 1)
        d['bqk_s_col'] = col_layout(np.concatenate([bqp, bkp]))
        d['wv_s'] = (1.0 + scale_msa[b])[:, None] * w_qkv_s[:, 2 * C:]
        d['bv_s_row'] = (shift_msa[b] @ w_qkv_s[:, 2 * C:] + b_qkv_s[2 * C:])[None]
        d['wproj_s'] = w_proj_s * gate_msa[b][None, :]
        d['bproj_s_row'] = (b_proj_s * gate_msa[b])[None]
        qe, qo = de(w_qkv_t[:, 0:C])
        ke, ko = de(w_qkv_t[:, C:2 * C])
        d['wqk_t_de'] = np.concatenate([tpad(qe), tpad(qo), tpad(ke), tpad(ko)], 1)
        bqe, bqo = de(b_qkv_t[0:C][None])
        bke, bko = de(b_qkv_t[C:2 * C][None])
        d['bqk_t_col'] = col_layout(np.concatenate(
            [tpad_vec(bqe[0]), tpad_vec(bqo[0]),
             tpad_vec(bke[0]), tpad_vec(bko[0])]))
        d['wv_t'] = w_qkv_t[:, 2 * C:]
        d['bv_t_row'] = b_qkv_t[2 * C:][None]
        d['wproj_t'] = w_proj_t * gate_msa[b][None, :]
        d['bproj_t_row'] = (b_proj_t * gate_msa[b])[None]
        Wqc_p, bqc_p = pad_heads(wq_c, bq_c, P)
        d['wq_c'] = Wqc_p
        d['bq_c_col'] = col_layout(bqc_p)
        kv = (y[b] @ wkv_c + bkv_c).reshape(LY, 2, NH, HD)
        k_c = kv[:, 0].reshape(LY, C)
        v_c = kv[:, 1].reshape(LY, C)
        kct = np.zeros((P, NH, LY), fnp)
        for h in range(NH):
            kct[:HD, h] = k_c[:, h * HD:(h + 1) * HD].T
        d['k_ct_pad'] = kct
        vca = np.zeros((LY, AF), fnp)
        for h in range(NH):
            vca[:, h * (HD + 1):h * (HD + 1) + HD] = v_c[:, h * HD:(h + 1) * HD]
            vca[:, h * (HD + 1) + HD] = 1.0
        d['v_c_aug'] = vca
        d['wo_c'] = wo_c
        d['bo_c_row'] = bo_c[None]
        d['w1'] = (1.0 + scale_mlp[b])[:, None] * w_fc1
        d['b1_col'] = col_layout(shift_mlp[b] @ w_fc1 + b_fc1)
        d['w2'] = w_fc2 * gate_mlp[b][None, :]
        d['b2_row'] = (b_fc2 * gate_mlp[b])[None]
        per_batch.append(d)

    cosT_in = np.ascontiguousarray(
        cosT.reshape(8, P, Q).transpose(1, 0, 2)).astype(bf16np)
    sinT_in = np.ascontiguousarray(
        sinT.reshape(8, P, Q).transpose(1, 0, 2)).astype(bf16np)
    mask_in = Mwin.astype(bf16np)

    in_maps = []
    for c in range(8):
        b, par = c // 2, c % 2
        d = per_batch[b]
        m = {}
        for k, v in d.items():
            if k.endswith('_col'):
                m[k] = np.ascontiguousarray(v, fnp)
            else:
                m[k] = np.ascontiguousarray(v).astype(bf16np)
        m['xa'] = np.ascontiguousarray(x[b, par * TOK:(par + 1) * TOK])
        m['cosT'] = cosT_in
        m['sinT'] = sinT_in
        m['mask'] = mask_in
        msel = np.zeros((P, 2), fnp)
        msel[:, 0] = 1.0 - par
        msel[:, 1] = par
        m['msel'] = msel
        in_maps.append(m)
    return in_maps


def kernel(**inputs):
    global _CACHED_NC
    if _CACHED_NC is None:
        _CACHED_NC = build_nc()
    in_maps = _host_precompute(inputs)
    res = run_bass_kernel_spmd(_CACHED_NC, in_maps, list(range(8)))
    out = np.zeros((B, N, C), np.float32)
    for c in range(8):
        b, par = c // 2, c % 2
        ob = res.results[c]["out"]
        out[b].reshape(T, S, C)[:, par * P:(par + 1) * P] = \
            ob.reshape(P, T, C).transpose(1, 0, 2)
    return out
